# revision 79
# baseline (speedup 1.0000x reference)
import math

import ml_dtypes
import numpy as np

import concourse.bacc as bacc
from concourse.bass import broadcast_tensor_aps
import concourse.mybir as mybir
import concourse.tile as tile
from concourse.bass_utils import run_bass_kernel_spmd

F32 = mybir.dt.float32
BF16 = mybir.dt.bfloat16
AF = mybir.ActivationFunctionType

S = 2048
B = 2
D = 1024
HD = 64
SCALE = 1.0 / np.sqrt(32.0)
MASK_VALUE = -1.0e6
ND = 8

_cached_nc = {}
LAST_RESULTS = None


def _interleave(primary, filler, weights=None):
    n = len(primary)
    if weights is None:
        weights = [1.0] * len(filler)
    total = sum(weights)
    out = []
    fi = 0
    acc = 0.0
    for i, u in enumerate(primary):
        out.append(u)
        want = (i + 1) * total / max(1, n)
        while fi < len(filler) and acc < want:
            out.append(filler[fi])
            acc += weights[fi]
            fi += 1
    out.extend(filler[fi:])
    return out


def _build(c0, c1, force_serial=False):
    CB = [c0, c1]
    KTOT = (c0 + c1) * 128
    KOFF = [0, c0 * 128]

    nc = bacc.Bacc("TRN2", target_bir_lowering=False, debug=False,
                   num_swdge_queues=4)

    xqT = nc.dram_tensor("xqT", [D, B * S], BF16, kind="ExternalInput")
    xkT = nc.dram_tensor("xkT", [D, KTOT], BF16, kind="ExternalInput")
    xvT = nc.dram_tensor("xvT", [D, KTOT], BF16, kind="ExternalInput")
    wq = nc.dram_tensor("wq", [D, 128], BF16, kind="ExternalInput")
    wk = nc.dram_tensor("wk", [D, 128], BF16, kind="ExternalInput")
    wv = nc.dram_tensor("wv", [D, 128], BF16, kind="ExternalInput")
    wo = nc.dram_tensor("wo", [128, D], BF16, kind="ExternalInput")
    maskb = nc.dram_tensor("maskb", [128, c0 + c1], F32, kind="ExternalInput")
    ident = nc.dram_tensor("ident", [128, 128], BF16, kind="ExternalInput")
    out = nc.dram_tensor("out", [B * S, D], BF16, kind="ExternalOutput")

    serial = force_serial or 4 * (c0 + c1) > 44
    ep_bufs = min(4 * (c0 + c1), 28) if not serial else 2 * max(c0, c1) + 4
    kv_segs = []
    for b in range(B):
        lo, hi = KOFF[b], KOFF[b] + CB[b] * 128
        for g0 in range(lo, hi, 512):
            kv_segs.append((g0, min(512, hi - g0)))
    q_segs = [(g, 1024) for g in range(0, B * S, 1024)]

    with tile.TileContext(nc) as tc:
        with tc.tile_pool(name="wp", bufs=1) as wp, \
             tc.tile_pool(name="per", bufs=1) as per, \
             tc.tile_pool(name="xkp", bufs=min(2, len(kv_segs)) * ND) as xkp, \
             tc.tile_pool(name="xvp", bufs=min(2, len(kv_segs)) * ND) as xvp, \
             tc.tile_pool(name="xqp", bufs=2 * ND) as xqp, \
             tc.tile_pool(name="ep", bufs=ep_bufs) as ep, \
             tc.tile_pool(name="ocp", bufs=2) as ocp, \
             tc.tile_pool(name="otp", bufs=4) as otp, \
             tc.tile_pool(name="smol", bufs=8) as smol, \
             tc.tile_pool(name="stp", bufs=4) as stp, \
             tc.tile_pool(name="pj", bufs=2, space="PSUM") as pj, \
             tc.tile_pool(name="psc", bufs=2, space="PSUM") as psc, \
             tc.tile_pool(name="pav", bufs=2, space="PSUM") as pav:

            mt = wp.tile([128, c0 + c1], F32, name="mt", tag="mt")
            nc.scalar.dma_start(out=mt, in_=maskb[:, :])
            idt = wp.tile([128, 128], BF16, name="idt", tag="idt")
            nc.scalar.dma_start(out=idt, in_=ident[:, :])
            wk_p = wp.tile([128, ND * 128], BF16, name="wk_p", tag="wk_p")
            wq_p = wp.tile([128, ND * 128], BF16, name="wq_p", tag="wq_p")
            wv_p = wp.tile([128, ND * 128], BF16, name="wv_p", tag="wv_p")
            nc.scalar.dma_start(out=wk_p.rearrange("p (n j) -> p n j", j=128),
                                in_=wk.rearrange("(n p) j -> p n j", p=128))
            nc.scalar.dma_start(out=wv_p.rearrange("p (n j) -> p n j", j=128),
                                in_=wv.rearrange("(n p) j -> p n j", p=128))
            wo_s = wp.tile([128, D], BF16, name="wo_s", tag="wo_s")
            ones = wp.tile([128, 1], BF16, name="ones", tag="ones")
            nc.vector.memset(ones, 1.0)
            scr1 = wp.tile([1, 1], F32, name="scr1", tag="scr1")
            nc.scalar.activation(scr1, mt[0:1, 0:1], AF.Exp)

            QTt = [per.tile([128, 1024], BF16, name=f"QT{t}", tag=f"QT{t}")
                   for t in range(4)]
            KTb = [per.tile([128, CB[b] * 128], BF16, name=f"KT{b}",
                            tag=f"KT{b}") for b in range(B)]
            Vn = [per.tile([128, 128], BF16, name=f"Vn{i}", tag=f"Vn{i}")
                  for i in range(c0 + c1)]
            recs = per.tile([128, B * 32], F32, name="recs", tag="recs")
            Et = {}

            def make_tiles(segs, nm, pool):
                return [(cc, w,
                         [pool.tile([128, w], BF16, name=f"{nm}{cc}_{d}",
                                    tag=f"xin_{nm}")
                          for d in range(ND)])
                        for cc, w in segs]

            def emit_dma(dram, tiles, seg_i, ds, eng):
                cc, w, row = tiles[seg_i]
                for d in ds:
                    eng.dma_start(out=row[d],
                                  in_=dram[d * 128:(d + 1) * 128, cc:cc + w])

            xk_t = make_tiles(kv_segs, "xk", xkp)
            xv_t = make_tiles(kv_segs, "xv", xvp)
            xq_t = make_tiles(q_segs, "xq", xqp)
            LOHI = (range(0, 4), range(4, ND))
            emit_dma(xkT, xk_t, 0, LOHI[0], nc.sync)
            nc.sync.dma_start(out=wq_p.rearrange("p (n j) -> p n j", j=128),
                              in_=wq.rearrange("(n p) j -> p n j", p=128))
            emit_dma(xvT, xv_t, 0, LOHI[0], nc.sync)
            for t in range(1, len(kv_segs)):
                emit_dma(xkT, xk_t, t, LOHI[0], nc.sync)
                emit_dma(xvT, xv_t, t, LOHI[0], nc.sync)
            nc.sync.dma_start(out=wo_s, in_=wo[:, :])
            emit_dma(xkT, xk_t, 0, LOHI[1], nc.gpsimd)
            emit_dma(xqT, xq_t, 0, LOHI[1], nc.gpsimd)
            emit_dma(xqT, xq_t, 0, LOHI[0], nc.gpsimd)
            emit_dma(xvT, xv_t, 0, LOHI[1], nc.gpsimd)
            emit_dma(xqT, xq_t, 1, range(ND), nc.gpsimd)
            for t in range(1, len(kv_segs)):
                emit_dma(xkT, xk_t, t, LOHI[1], nc.gpsimd)
                emit_dma(xvT, xv_t, t, LOHI[1], nc.gpsimd)
            for g in range(2, len(q_segs)):
                emit_dma(xqT, xq_t, g, range(ND), nc.gpsimd)

            wk_t = [wk_p[:, d * 128:(d + 1) * 128] for d in range(ND)]
            wq_t = [wq_p[:, d * 128:(d + 1) * 128] for d in range(ND)]
            wv_t = [wv_p[:, d * 128:(d + 1) * 128] for d in range(ND)]

            def x_cols(tiles, lo, hi):
                for cc, w, row in tiles:
                    if cc <= lo and hi <= cc + w:
                        return [r[:, lo - cc:hi - cc] for r in row]
                raise AssertionError((lo, hi))

            def k_proj(b):
                lo, hi = KOFF[b], KOFF[b] + CB[b] * 128
                for g0 in range(lo, hi, 512):
                    g1 = min(g0 + 512, hi)
                    xr = x_cols(xk_t, g0, g1)
                    pt = pj.tile([128, g1 - g0], F32, name=f"pk{g0}", tag="pj")
                    for d in range(ND):
                        nc.tensor.matmul(pt, wk_t[d], xr[d],
                                         start=(d == 0), stop=(d == ND - 1))
                    nc.vector.tensor_copy(KTb[b][:, g0 - lo:g1 - lo], pt)

            def q_proj(r):
                xr = x_cols(xq_t, r * 512, (r + 1) * 512)
                pt = pj.tile([128, 512], F32, name=f"pq{r}", tag="pj")
                for d in range(ND):
                    nc.tensor.matmul(pt, wq_t[d], xr[d],
                                     start=(d == 0), stop=(d == ND - 1))
                nc.vector.tensor_copy(
                    QTt[r // 2][:, (r % 2) * 512:(r % 2) * 512 + 512], pt)

            def v_proj(b, kc):
                lo = KOFF[b] + kc * 128
                xr = x_cols(xv_t, lo, lo + 128)
                pt = pj.tile([128, 128], F32, name=f"pv{lo}", tag="pj")
                for d in range(ND):
                    nc.tensor.matmul(pt, xr[d], wv_t[d],
                                     start=(d == 0), stop=(d == ND - 1))
                nc.vector.tensor_copy(Vn[KOFF[b] // 128 + kc], pt)

            def s_unit(b, kc, h, qh):
                hoff = h * 64
                et = ep.tile([128, 1024], BF16, name=f"et{b}_{h}_{kc}_{qh}",
                             tag="et")
                Et[(b, h, kc, qh)] = et
                pst = psc.tile([128, 1024], F32,
                               name=f"pst{b}_{h}_{kc}_{qh}", tag="pst")
                for qg in range(2):
                    nc.tensor.matmul(
                        pst[:, qg * 512:(qg + 1) * 512],
                        KTb[b][hoff:hoff + 64, kc * 128:(kc + 1) * 128],
                        QTt[b * 2 + qh][hoff:hoff + 64,
                                        qg * 512:(qg + 1) * 512],
                        start=True, stop=True)
                nc.scalar.activation(
                    et, pst, AF.Exp,
                    bias=mt[:, KOFF[b] // 128 + kc:KOFF[b] // 128 + kc + 1],
                    scale=float(SCALE))

            copy_rr = [0]

            def psum_out_copy(dst, src, late):
                k = copy_rr[0] % 4
                copy_rr[0] += 1
                if (late and k != 3) or (not late and k == 1):
                    nc.scalar.copy(dst, src)
                else:
                    nc.vector.tensor_copy(dst, src)

            pots = {}
            ocss = {}

            def bq_av(b, qq):
                cb = CB[b]
                qh = qq // 2
                pots[(b, qq)] = [
                    pav.tile([128, 260], F32, name=f"po{b}_{qq}_{h}",
                             tag="pav")
                    for h in range(2)]
                for h in range(2):
                    pt = pots[(b, qq)][h]
                    for i in range(4):
                        qs = (qq % 2) * 4 + i
                        for ci in range(cb):
                            nc.tensor.matmul(
                                pt[:, i * 64:(i + 1) * 64],
                                Et[(b, h, ci, qh)][:, qs * 128:(qs + 1) * 128],
                                Vn[KOFF[b] // 128 + ci][:, h * 64:h * 64 + 64],
                                start=(ci == 0),
                                stop=(ci == cb - 1),
                                skip_group_check=True)
                    for i in range(4):
                        qs = (qq % 2) * 4 + i
                        for ci in range(cb):
                            nc.tensor.matmul(
                                pt[:, 256 + i:257 + i],
                                Et[(b, h, ci, qh)][:, qs * 128:(qs + 1) * 128],
                                ones, start=(ci == 0), stop=(ci == cb - 1),
                                skip_group_check=True)

            def bq_nm(b, qq):
                rcol = (b * 4 + qq) * 8
                dn = smol.tile([128, 8], F32, name=f"dn{b}_{qq}", tag="dn")
                for h in range(2):
                    nc.vector.tensor_copy(dn[:, h * 4:(h + 1) * 4],
                                          pots[(b, qq)][h][:, 256:260])
                rscr = smol.tile([128, 8], F32, name=f"rs{b}_{qq}", tag="rs")
                nc.vector.reciprocal_approx_accurate(
                    recs[:, rcol:rcol + 8], dn, rscr)
                ocs = ocp.tile([128, 512], BF16, name=f"oc{b}_{qq}", tag="ocs")
                oc3 = ocs.rearrange("p (a c) -> p a c", c=128)
                for h in range(2):
                    p3 = pots[(b, qq)][h][:, 0:256].rearrange(
                        "p (a b) -> p a b", b=64)
                    r3 = recs[:, rcol + h * 4:rcol + h * 4 + 4].rearrange(
                        "p (a b) -> p a b", b=1)
                    rb, pb = broadcast_tensor_aps(r3, p3)
                    nc.vector.tensor_mul(
                        oc3[:, :, h * 64:h * 64 + 64], pb, rb)
                ocss[(b, qq)] = ocs

            def bq_ow(b, qq, tail=False):
                ocs = ocss[(b, qq)]
                tt = pav.tile([128, 512], BF16, name=f"tt{b}_{qq}", tag="pav")
                for i in range(4):
                    nc.tensor.matmul(tt[:, i * 128:(i + 1) * 128],
                                     ocs[:, i * 128:(i + 1) * 128], idt,
                                     is_transpose=True, start=(i == 0),
                                     stop=True, skip_group_check=True)
                ott = otp.tile([128, 512], BF16, name=f"ot{b}_{qq}", tag="ot")
                nc.vector.tensor_copy(ott, tt)
                for i in range(4):
                    qs = qq * 4 + i
                    ot = ott[:, i * 128:(i + 1) * 128]
                    row = b * S + qs * 128
                    st = stp.tile([128, 1024], BF16, name=f"st{b}_{qs}",
                                  tag="st")
                    for ec in range(2):
                        pw = pj.tile([128, 512], F32, name=f"pw{b}_{qs}_{ec}",
                                     tag="pj")
                        nc.tensor.matmul(pw, ot,
                                         wo_s[:, ec * 512:(ec + 1) * 512],
                                         start=True, stop=True)
                        psum_out_copy(st[:, ec * 512:(ec + 1) * 512], pw,
                                      late=(b == 1 and qq >= 1))
                        if tail:
                            eng = nc.sync if ec == 0 else nc.gpsimd
                            eng.dma_start(
                                out=out[row:row + 128,
                                        ec * 512:(ec + 1) * 512],
                                in_=st[:, ec * 512:(ec + 1) * 512])
                    if not tail:
                        nc.sync.dma_start(out=out[row:row + 128, :], in_=st)

            U = lambda f, *a: (lambda: f(*a))
            s_units = {
                (b, qh): [U(s_unit, b, kc, h, qh)
                          for kc in range(CB[b]) for h in range(2)]
                for b in range(B) for qh in range(2)
            }

            if not serial:
                k_proj(0)
                q_proj(0)
                q_proj(1)
                for u in _interleave(
                        s_units[(0, 0)],
                        [U(v_proj, 0, kc) for kc in range(c0)]
                        + [U(q_proj, 2), U(q_proj, 3)]):
                    u()
                for u in _interleave(
                        s_units[(0, 1)],
                        [U(k_proj, 1), U(q_proj, 4), U(q_proj, 5)]
                        + [U(v_proj, 1, kc) for kc in range(min(3, c1))]):
                    u()
                for u in _interleave(
                        s_units[(1, 0)],
                        [U(bq_av, 0, 0), U(bq_nm, 0, 0), U(bq_ow, 0, 0)]
                        + [U(v_proj, 1, kc) for kc in range(3, c1)]
                        + [U(q_proj, 6), U(q_proj, 7)]
                        + [U(bq_av, 0, 1), U(bq_nm, 0, 1), U(bq_ow, 0, 1)]):
                    u()
                filler4 = [U(bq_av, 0, 2), U(bq_nm, 0, 2), U(bq_ow, 0, 2),
                           U(bq_av, 0, 3), U(bq_nm, 0, 3), U(bq_ow, 0, 3),
                           U(bq_av, 1, 0), U(bq_nm, 1, 0),
                           U(bq_av, 1, 1), U(bq_nm, 1, 1), U(bq_ow, 1, 0)]
                for u in _interleave(s_units[(1, 1)], filler4):
                    u()
                bq_ow(1, 1, tail=True)
                bq_av(1, 2)
                bq_nm(1, 2)
                bq_av(1, 3)
                bq_nm(1, 3)
                bq_ow(1, 2, tail=True)
                bq_ow(1, 3, tail=True)
            else:
                k_proj(0)
                k_proj(1)
                for r in range(8):
                    q_proj(r)
                for b in range(B):
                    for kc in range(CB[b]):
                        v_proj(b, kc)
                for b in range(B):
                    for qh in range(2):
                        for u in s_units[(b, qh)]:
                            u()
                        for qq in (qh * 2, qh * 2 + 1):
                            bq_av(b, qq)
                            bq_nm(b, qq)
                            bq_ow(b, qq, tail=(b == 1 and qq == 3))
    nc.compile()
    return nc


def _get_nc(c0, c1):
    key = (c0, c1)
    if key not in _cached_nc:
        try:
            _cached_nc[key] = _build(c0, c1)
        except Exception:
            if (c0, c1) == (3, 5):
                raise
            _cached_nc[key] = _build(c0, c1, force_serial=True)
    return _cached_nc[key]


def kernel(queries, keys, values, valid_lens, Wq, Wk, Wv, Wo, **kwargs):
    queries = np.asarray(queries, dtype=np.float32)
    keys = np.asarray(keys, dtype=np.float32)
    values = np.asarray(values, dtype=np.float32)
    Wq = np.asarray(Wq, dtype=np.float32)
    Wk = np.asarray(Wk, dtype=np.float32)
    Wv = np.asarray(Wv, dtype=np.float32)
    Wo = np.asarray(Wo, dtype=np.float32)
    vls = np.asarray(valid_lens).astype(np.int64)
    assert queries.shape == (B, S, D), \
        f"kernel compiled for ({B}, {S}, {D}), got {queries.shape}"

    bf16 = ml_dtypes.bfloat16
    cs = [max(1, min(16, int(math.ceil(int(vls[b]) / 128)))) for b in range(B)]
    c0, c1 = cs
    nc = _get_nc(c0, c1)

    xq = np.ascontiguousarray(
        np.concatenate([queries[0].T, queries[1].T], axis=1)).astype(bf16)
    xk = np.ascontiguousarray(np.concatenate(
        [keys[0][:c0 * 128].T, keys[1][:c1 * 128].T], axis=1)).astype(bf16)
    xv = np.ascontiguousarray(np.concatenate(
        [values[0][:c0 * 128].T, values[1][:c1 * 128].T], axis=1)).astype(bf16)
    mk = []
    for b in range(B):
        vl = int(vls[b])
        kidx = np.arange(cs[b] * 128)
        mb = np.where(kidx < vl, 0.0, MASK_VALUE).astype(np.float32)
        mk.append(mb.reshape(cs[b], 128).T)
    mkt = np.ascontiguousarray(np.concatenate(mk, axis=1))
    idn = np.eye(128, dtype=bf16)

    in_maps = []
    for g in range(8):
        in_maps.append({
            "xqT": xq, "xkT": xk, "xvT": xv,
            "wq": np.ascontiguousarray(Wq[:, g * 128:(g + 1) * 128]).astype(bf16),
            "wk": np.ascontiguousarray(Wk[:, g * 128:(g + 1) * 128]).astype(bf16),
            "wv": np.ascontiguousarray(Wv[:, g * 128:(g + 1) * 128]).astype(bf16),
            "wo": np.ascontiguousarray(Wo[g * 128:(g + 1) * 128, :]).astype(bf16),
            "maskb": mkt, "ident": idn,
        })

    res = run_bass_kernel_spmd(nc, in_maps, core_ids=list(range(8)), **kwargs)
    global LAST_RESULTS
    LAST_RESULTS = res

    acc = res.results[0]["out"].astype(np.float32)
    for g in range(1, 8):
        acc = acc + res.results[g]["out"].astype(np.float32)
    return np.ascontiguousarray(acc.reshape(B, S, D))
